# revision 6
# baseline (speedup 1.0000x reference)
"""Trainium2 Bass kernel for nn_DirectInjectionEncoder (moe_routing).

Strategy (8 NeuronCores):
  - Every projection GEMM (Wgate/Wup/Wdown 10240->2560 and Wk/Wv 640->2560)
    is sharded over the output dim d_model=2560 -> 320 columns per core, so
    each core streams only 1/8 of every weight from HBM.
  - Big-group GEMMs run as e4m3 DoubleRow matmuls (2x PE rate, both
    operands fp8, K=256 per instruction). A KERNEL_F8_FRAC knob can move a
    fraction of the contraction to e3m4-stationary x bf16-moving matmuls
    (1x PE rate, higher precision) inside the same PSUM accumulation
    group. Weights are pre-scaled by 50 on the host; the row normalization
    divides the scale back out. Row norms come from fp32 PSUM
    square-accumulation, so fp8 only affects the GEMM itself.
  - The DoubleRow path streams x and W together: the host packs
    [x-rows | W-cols] into one supertile per k-chunk so each chunk is a
    single large contiguous DMA. Chunk sizes are graduated (2,6,8,...) so
    the PE's first matmul has data as early as possible.
  - Row L2-norms need the full 2560-dim row: each core computes partial
    sums of squares; two small AllReduces (~19us fixed cost each on this
    8-core mesh) combine them. AR1 covers the first two big groups and
    flies while the rest computes; AR2 covers the last big group + both kv
    groups (ordered last because they are tiny), so only one collective
    sits in the tail.
  - Identity tokens (9 of 14 slots/layer, first 2560 dims, no weights) are
    data-parallel over the batch in bf16 both ways; their loads/stores and
    ACT/DVE norm work fill PE-idle and collective windows.
  - All outputs are stored as bf16 (the 2e-2 rel-err budget dwarfs bf16
    rounding); the host casts back to fp32 when assembling.
"""

import os
import sys

sys.path.insert(0, "/opt/trn_rl_repo")

import numpy as np
import ml_dtypes

from concourse import bacc, mybir
from concourse.bass_utils import run_bass_kernel_spmd
from concourse.tile import TileContext

D_MODEL = 2560
NUM_LAYERS = 36
TOKENS_PER_LAYER = 14
EPS = 1e-8
B = 16
N_CORES = 8
CORE_IDS = list(range(N_CORES))
D_SHARD = D_MODEL // N_CORES  # 320
ROWS = B * NUM_LAYERS  # 576
XW = ROWS + D_SHARD  # 896 combined columns in the fused x|w supertiles
ROW_TILES = [(0, 128), (128, 128), (256, 128), (384, 128), (512, 64)]
NRT = len(ROW_TILES)

IDENTITY_OFFSETS = np.array([0, 1, 2, 4, 6, 7, 8, 10, 13])
# big groups first; tiny kv groups last so the tail collective covers the
# least-latency-critical work
GROUP_ORDER = [
    ("big", 11, "Wup"),
    ("big", 9, "Wgate"),
    ("big", 12, "Wdown"),
    ("kv", 3, "Wk"),
    ("kv", 5, "Wv"),
]
KV_IND = 640
BIG_IND = 10240
N_SSQ = 25  # 5 groups x 5 row tiles, in GROUP_ORDER order
AR1_COLS = 10  # big0 + big1

ID_ROWS_RAW = (B // N_CORES) * NUM_LAYERS * len(IDENTITY_OFFSETS)  # 648
ID_SUP = 2  # two supertiles of 3 row-blocks each (768 rows padded)

W_SCALE = 50.0  # folded into the normalization

F32 = mybir.dt.float32
BF16 = mybir.dt.bfloat16
F8E4 = mybir.dt.float8e4
F8E3 = mybir.dt.float8e3
DR = mybir.MatmulPerfMode.DoubleRow
AF = mybir.ActivationFunctionType

NP_BF16 = ml_dtypes.bfloat16
NP_E4 = ml_dtypes.float8_e4m3
NP_E3 = ml_dtypes.float8_e3m4

# fraction of the big-group contraction dim run as e4m3 DoubleRow
F8_FRAC = float(os.environ.get("KERNEL_F8_FRAC", "1.0"))
N_KT = BIG_IND // 128  # 80 contraction tiles of 128
N_DR = int(round(F8_FRAC * N_KT / 2))  # 256-wide DoubleRow k-tiles
N_E3 = N_KT - 2 * N_DR  # 128-wide e3m4xbf16 k-tiles
K_DR = 256 * N_DR


def _dr_chunks(n):
    # graduated chunk sizes: small first chunks cut PE startup latency
    out = []
    for c in (2, 6):
        if n >= c:
            out.append(c)
            n -= c
    while n > 8:
        out.append(8)
        n -= 8
    if n:
        out.append(n)
    return out


CHUNKS_DR = _dr_chunks(N_DR) if N_DR else []


def _pick_kb(n, cap=8):
    for kb in range(min(n, cap), 0, -1):
        if n % kb == 0:
            return kb
    return 1


KB_E3 = _pick_kb(N_E3, 8) if N_E3 else 1
NSUP_E3 = N_E3 // KB_E3 if N_E3 else 0


def _positions(offset):
    return np.arange(NUM_LAYERS) * TOKENS_PER_LAYER + offset


def build_program():
    nc = bacc.Bacc("TRN2", num_devices=N_CORES)

    big_idx = [i for i, g in enumerate(GROUP_ORDER) if g[0] == "big"]
    kv_idx = [i for i, g in enumerate(GROUP_ORDER) if g[0] == "kv"]

    xw_d, xe3_d, we3_d, om_d = {}, {}, {}, {}
    for gi in big_idx:
        for ci, kb in enumerate(CHUNKS_DR):
            xw_d[(gi, ci)] = nc.declare_dram_parameter(f"xw_{gi}_{ci}", [128, kb, 2, XW], F8E4, isOutput=False)
        if NSUP_E3:
            xe3_d[gi] = nc.declare_dram_parameter(f"xe3_{gi}", [NSUP_E3, 128, KB_E3, ROWS], F8E3, isOutput=False)
            we3_d[gi] = nc.declare_dram_parameter(f"we3_{gi}", [NSUP_E3, 128, KB_E3, D_SHARD], BF16, isOutput=False)
        om_d[gi] = nc.declare_dram_parameter(f"om_{gi}", [128, NRT * D_SHARD], BF16, isOutput=True)
    kvx_d, kvw_d, kvo_d = {}, {}, {}
    for gi in kv_idx:
        kvx_d[gi] = nc.declare_dram_parameter(f"kvx_{gi}", [128, 5, ROWS], BF16, isOutput=False)
        kvw_d[gi] = nc.declare_dram_parameter(f"kvw_{gi}", [128, 5, D_SHARD], BF16, isOutput=False)
        kvo_d[gi] = nc.declare_dram_parameter(f"kvo_{gi}", [128, NRT * D_SHARD], BF16, isOutput=True)
    idx_d = nc.declare_dram_parameter("id_x", [ID_SUP, 128, 3 * D_MODEL], BF16, isOutput=False)
    ido_d = nc.declare_dram_parameter("out_id", [ID_SUP, 128, 3 * D_MODEL], BF16, isOutput=True)

    with TileContext(nc) as tc:
        with (
            tc.tile_pool(name="xw", bufs=3) as xw_pool,
            tc.tile_pool(name="xe", bufs=3) as xe_pool,
            tc.tile_pool(name="we", bufs=3) as we_pool,
            tc.tile_pool(name="sout", bufs=N_SSQ) as sout_pool,
            tc.tile_pool(name="scr", bufs=2) as scr_pool,
            tc.tile_pool(name="kvp", bufs=4) as kv_pool,
            tc.tile_pool(name="idp", bufs=2) as id_pool,
            tc.tile_pool(name="obf", bufs=6) as obf_pool,
            tc.tile_pool(name="small", bufs=1) as small_pool,
            tc.tile_pool(name="ps", bufs=8, space="PSUM") as psum_pool,
            tc.tile_pool(name="dram", bufs=1, space="DRAM") as dram_pool,
        ):
            ssq = small_pool.tile([128, N_SSQ], F32, tag="ssq")
            nc.vector.memset(ssq[:], 0.0)

            # Warmup AllReduce: the first collective pays a ~35us all-core
            # barrier plus ~30us cold cost; hide both under the GEMM phase.
            warm_sb = small_pool.tile([1, 16], F32, tag="warmsb")
            nc.vector.memset(warm_sb[:], 0.0)
            warm_buf = dram_pool.tile([16], F32, tag="warmci")
            nc.gpsimd.dma_start(out=warm_buf[:], in_=warm_sb[0, :])
            nc.gpsimd.collective_compute(
                "AllReduce",
                mybir.AluOpType.add,
                ins=[warm_buf.opt()],
                outs=[warm_buf.opt()],
                replica_groups=[CORE_IDS],
            )

            id_tiles = [None] * ID_SUP
            id_out = [None] * ID_SUP
            id_ssq = small_pool.tile([128, ID_SUP * 3], F32, tag="idssq")
            id_scale = small_pool.tile([128, ID_SUP * 3], F32, tag="idscale")

            def id_load(s):
                it = id_pool.tile([128, 3, D_MODEL], BF16, tag="idin", name=f"idin_{s}")
                nc.sync.dma_start(out=it[:], in_=idx_d[s].rearrange("p (j c) -> p j c", j=3))
                id_tiles[s] = it
                id_out[s] = id_pool.tile([128, 3, D_MODEL], BF16, tag="idout", name=f"idout_{s}")

            def id_square(s, j):
                iscr = scr_pool.tile([128, D_MODEL], BF16, tag="idscr", name=f"idscr_{s}_{j}")
                nc.scalar.activation(
                    iscr[:], id_tiles[s][:, j, :], AF.Square,
                    accum_out=id_ssq[:, s * 3 + j : s * 3 + j + 1],
                )

            def id_finish(s):
                c0 = s * 3
                nrm = small_pool.tile([128, 3], F32, tag=f"idn{s}", name=f"idnrm_{s}")
                nc.scalar.sqrt(nrm[:], id_ssq[:, c0 : c0 + 3])
                nc.scalar.activation(nrm[:], nrm[:], AF.Copy, bias=EPS)
                nc.vector.reciprocal(id_scale[:, c0 : c0 + 3], nrm[:])
                for j in range(3):
                    nc.vector.tensor_scalar_mul(
                        id_out[s][:, j, :], id_tiles[s][:, j, :],
                        id_scale[:, c0 + j : c0 + j + 1],
                    )
                nc.scalar.dma_start(
                    out=ido_d[s].rearrange("p (j c) -> p j c", j=3), in_=id_out[s][:]
                )

            kv_tiles = {}

            def kv_load():
                for gi in kv_idx:
                    kvx = kv_pool.tile([128, 5, ROWS], BF16, tag="kvx", name=f"kvx_{gi}")
                    kvw = kv_pool.tile([128, 5, D_SHARD], BF16, tag="kvw", name=f"kvw_{gi}")
                    nc.sync.dma_start(out=kvx[:], in_=kvx_d[gi][:, :, :])
                    nc.sync.dma_start(out=kvw[:], in_=kvw_d[gi][:, :, :])
                    kv_tiles[gi] = (kvx, kvw)

            souts = {}

            def big_group(gi):
                ps = [
                    psum_pool.tile([128, D_SHARD], F32, tag="ps", name=f"ps_{gi}_{r}")
                    for r in range(NRT)
                ]
                kt = 0
                for ci, kb in enumerate(CHUNKS_DR):
                    xw = xw_pool.tile([128, kb, 2, XW], F8E4, tag="xw", name=f"xw_{gi}_{ci}")
                    nc.sync.dma_start(out=xw[:], in_=xw_d[(gi, ci)][:, :, :, :])
                    for j in range(kb):
                        for r, (r0, rw) in enumerate(ROW_TILES):
                            nc.tensor.matmul(
                                ps[r][:rw, :],
                                xw[:, j, :, r0 : r0 + rw],
                                xw[:, j, :, ROWS:],
                                start=(kt == 0),
                                stop=(N_E3 == 0 and kt == N_DR - 1),
                                perf_mode=DR,
                                skip_group_check=True,
                            )
                        kt += 1
                ke = 0
                for js in range(NSUP_E3):
                    xt = xe_pool.tile([128, KB_E3, ROWS], F8E3, tag="xe", name=f"xe3_{gi}_{js}")
                    wt = we_pool.tile([128, KB_E3, D_SHARD], BF16, tag="we", name=f"we3_{gi}_{js}")
                    nc.sync.dma_start(out=xt[:], in_=xe3_d[gi][js])
                    nc.sync.dma_start(out=wt[:], in_=we3_d[gi][js])
                    for j in range(KB_E3):
                        last = ke == N_E3 - 1
                        for r, (r0, rw) in enumerate(ROW_TILES):
                            nc.tensor.matmul(
                                ps[r][:rw, :],
                                xt[:, j, r0 : r0 + rw],
                                wt[:, j, :],
                                start=(N_DR == 0 and ke == 0),
                                stop=last,
                                skip_group_check=True,
                            )
                        ke += 1
                return ps

            def kv_group(gi):
                kvx, kvw = kv_tiles[gi]
                ps = [
                    psum_pool.tile([128, D_SHARD], F32, tag="ps", name=f"pkv_{gi}_{r}")
                    for r in range(NRT)
                ]
                for k in range(5):
                    for r, (r0, rw) in enumerate(ROW_TILES):
                        nc.tensor.matmul(
                            ps[r][:rw, :],
                            kvx[:, k, r0 : r0 + rw],
                            kvw[:, k, :],
                            start=(k == 0),
                            stop=(k == 4),
                        )
                return ps

            def drain(slot, ps):
                # copy psum->bf16 sbuf + square-accum into ssq columns
                for r, (r0, rw) in enumerate(ROW_TILES):
                    col = slot * NRT + r
                    scr = scr_pool.tile([128, D_SHARD], BF16, tag="scr", name=f"scr_{slot}_{r}")
                    nc.scalar.activation(
                        scr[:rw, :], ps[r][:rw, :], AF.Square,
                        accum_out=ssq[:rw, col : col + 1],
                    )
                    so = sout_pool.tile([128, D_SHARD], BF16, tag="sout", name=f"so_{slot}_{r}")
                    nc.vector.tensor_copy(so[:rw, :], ps[r][:rw, :])
                    souts[(slot, r)] = so

            def scale_store(slot, sc, sc_col0):
                kind = GROUP_ORDER[slot][0]
                ob = obf_pool.tile([128, NRT, D_SHARD], BF16, tag="obf", name=f"obf_{slot}")
                for r, (r0, rw) in enumerate(ROW_TILES):
                    col = slot * NRT + r - sc_col0
                    src = souts[(slot, r)]
                    if r % 2 == 0:
                        nc.vector.tensor_scalar_mul(
                            ob[:rw, r, :], src[:rw, :], sc[:rw, col : col + 1]
                        )
                    else:
                        nc.scalar.activation(
                            ob[:rw, r, :], src[:rw, :], AF.Copy,
                            scale=sc[:rw, col : col + 1],
                        )
                dst = om_d[slot] if kind == "big" else kvo_d[slot]
                nc.scalar.dma_start(
                    out=dst.rearrange("p (r c) -> p r c", r=NRT), in_=ob[:]
                )

            def reduce_scale(ar_buf, ncols, col0):
                # readback of the AllReduce result on the gpsimd ring, then
                # norm math on ACT/DVE
                tsq = small_pool.tile([128, ncols], F32, tag=f"tsq{col0}", name=f"tsq_{col0}")
                nc.gpsimd.dma_start(out=tsq[:], in_=ar_buf[:])
                nrm = small_pool.tile([128, ncols], F32, tag=f"nrm{col0}", name=f"nrm_{col0}")
                nc.scalar.sqrt(nrm[:], tsq[:])
                nc.scalar.activation(nrm[:], nrm[:], AF.Copy, bias=EPS)
                sc = small_pool.tile([128, ncols], F32, tag=f"sc{col0}", name=f"sc_{col0}")
                nc.vector.reciprocal(sc[:], nrm[:])
                return sc

            def all_reduce(cols0, cols1, tag):
                buf = dram_pool.tile([128, cols1 - cols0], F32, tag=f"ar{tag}")
                nc.gpsimd.dma_start(out=buf[:], in_=ssq[:, cols0:cols1])
                nc.gpsimd.collective_compute(
                    "AllReduce",
                    mybir.AluOpType.add,
                    ins=[buf.opt()],
                    outs=[buf.opt()],
                    replica_groups=[CORE_IDS],
                )
                return buf

            # ---- main pipeline ----
            ar1 = None
            for slot, (kind, off, wname) in enumerate(GROUP_ORDER):
                ps = big_group(slot) if kind == "big" else kv_group(slot)
                drain(slot, ps)
                if slot == 0:
                    id_load(0)
                    id_square(0, 0)
                    id_square(0, 1)
                    id_square(0, 2)
                    id_finish(0)
                elif slot == 1:
                    # AR1: big0 + big1 norm partials fly during big2/kv
                    ar1 = all_reduce(0, AR1_COLS, "1")
                    kv_load()
                    id_load(1)
                    id_square(1, 0)
                    id_square(1, 1)
                    id_square(1, 2)
                    id_finish(1)
                elif slot == 2:
                    # AR1 is long done by the time ACT reaches this point;
                    # emitting the norm math here keeps it ahead of the kv
                    # squares in the ACT queue.
                    sc1 = reduce_scale(ar1, AR1_COLS, 0)
                    scale_store(0, sc1, 0)
                    scale_store(1, sc1, 0)

            ar2 = all_reduce(AR1_COLS, N_SSQ, "2")

            sc2 = reduce_scale(ar2, N_SSQ - AR1_COLS, AR1_COLS)
            for slot in range(2, 5):
                scale_store(slot, sc2, AR1_COLS)

    nc.compile()
    return nc


_NC = None


def _get_nc():
    global _NC
    if _NC is None:
        _NC = build_program()
    return _NC


def _pack_sup_e3(xT):
    # [K, C] -> [nsup, 128, kb, C]; k = ((js*kb + j)*128 + p)
    K, C = xT.shape
    nsup = K // (128 * KB_E3)
    return np.ascontiguousarray(xT.reshape(nsup, KB_E3, 128, C).transpose(0, 2, 1, 3))


def _pack_chunks_dr(xwT):
    # [K_DR, 896] -> per-chunk [128, kb, 2, 896]; k = ((base + j)*2 + i)*128 + p
    out = []
    base = 0
    for kb in CHUNKS_DR:
        a = xwT[base * 256 : (base + kb) * 256]
        out.append(np.ascontiguousarray(a.reshape(kb, 2, 128, XW).transpose(2, 0, 1, 3)))
        base += kb
    return out


def _pack_kv(xT):
    # [640, C] -> [128, 5, C]
    K, C = xT.shape
    return np.ascontiguousarray(xT.reshape(5, 128, C).transpose(1, 0, 2))


def _prep_inputs(lora_tokens, weights):
    lora = np.ascontiguousarray(lora_tokens)
    big_idx = [(i, g[1], g[2]) for i, g in enumerate(GROUP_ORDER) if g[0] == "big"]
    kv_idx = [(i, g[1], g[2]) for i, g in enumerate(GROUP_ORDER) if g[0] == "kv"]

    shared = {}
    big_x = {}
    for gi, off, wname in big_idx:
        pos = _positions(off)
        x = lora[:, pos, :].reshape(ROWS, BIG_IND).T  # [10240, 576]
        big_x[gi] = x
        if N_E3:
            shared[f"xe3_{gi}"] = _pack_sup_e3(np.clip(x[K_DR:] * 2.0, -15.0, 15.0).astype(NP_E3))
    kv_xt = {}
    for gi, off, wname in kv_idx:
        pos = _positions(off)
        kv_xt[gi] = lora[:, pos, :KV_IND].reshape(ROWS, KV_IND).T  # [640, 576]
        shared[f"kvx_{gi}"] = _pack_kv(kv_xt[gi].astype(NP_BF16))

    id_pos = np.sort(np.concatenate([_positions(o) for o in IDENTITY_OFFSETS]))
    bpc = B // N_CORES
    in_maps = []
    for c in range(N_CORES):
        m = dict(shared)
        csl = slice(c * D_SHARD, (c + 1) * D_SHARD)
        for gi, off, wname in big_idx:
            wT = weights[wname][csl, :].T  # [10240, 320]
            if N_DR:
                xw = np.concatenate(
                    [
                        np.clip(big_x[gi][:K_DR], -240, 240),
                        np.clip(wT[:K_DR] * W_SCALE, -240, 240),
                    ],
                    axis=1,
                ).astype(NP_E4)
                for ci, chunk in enumerate(_pack_chunks_dr(xw)):
                    m[f"xw_{gi}_{ci}"] = chunk
            if N_E3:
                m[f"we3_{gi}"] = _pack_sup_e3((wT[K_DR:] * (W_SCALE / 2.0)).astype(NP_BF16))
        for gi, off, wname in kv_idx:
            m[f"kvw_{gi}"] = _pack_kv(weights[wname][csl, :].T.astype(NP_BF16))
        idx = lora[c * bpc : (c + 1) * bpc, :, :][:, id_pos, :D_MODEL].reshape(
            ID_ROWS_RAW, D_MODEL
        )
        idp = np.ones((ID_SUP * 3 * 128, D_MODEL), dtype=np.float32)
        idp[:ID_ROWS_RAW] = idx
        # row = (s*3 + j)*128 + p  ->  [ID_SUP, 128, 3*D_MODEL]
        m["id_x"] = np.ascontiguousarray(
            idp.reshape(ID_SUP, 3, 128, D_MODEL).transpose(0, 2, 1, 3).reshape(
                ID_SUP, 128, 3 * D_MODEL
            ).astype(NP_BF16)
        )
        in_maps.append(m)
    return in_maps, id_pos


def _unpack_rows(arr128, nrt=NRT, width=D_SHARD):
    # [128, nrt*width] (bf16) -> [nrt*128, width] fp32, caller trims rows
    a = np.asarray(arr128).astype(np.float32).reshape(128, nrt, width)
    return a.transpose(1, 0, 2).reshape(nrt * 128, width)


def run(inputs, trace=False):
    nc = _get_nc()
    weights = {k: inputs[k] for k in ("Wk", "Wv", "Wgate", "Wup", "Wdown")}
    in_maps, id_pos = _prep_inputs(inputs["lora_tokens"], weights)
    res = run_bass_kernel_spmd(nc, in_maps, CORE_IDS, trace=trace)

    out = np.zeros((B, NUM_LAYERS * TOKENS_PER_LAYER, D_MODEL), dtype=np.float32)
    bpc = B // N_CORES
    for c in range(N_CORES):
        r = res.results[c]
        csl = slice(c * D_SHARD, (c + 1) * D_SHARD)
        for slot, (kind, off, wname) in enumerate(GROUP_ORDER):
            pos = _positions(off)
            key = f"om_{slot}" if kind == "big" else f"kvo_{slot}"
            rows = _unpack_rows(r[key])[:ROWS]
            out[:, pos, csl] = rows.reshape(B, NUM_LAYERS, D_SHARD)
        ido = np.asarray(r["out_id"]).astype(np.float32).reshape(ID_SUP, 128, 3, D_MODEL)
        ido = ido.transpose(0, 2, 1, 3).reshape(ID_SUP * 3 * 128, D_MODEL)[:ID_ROWS_RAW]
        out[c * bpc : (c + 1) * bpc, id_pos, :] = ido.reshape(bpc, len(id_pos), D_MODEL)
    return out, res


def kernel(**inputs) -> np.ndarray:
    out, _ = run(inputs, trace=False)
    return out


# revision 7
# speedup vs baseline: 1.1991x; 1.1991x over previous
"""Trainium2 Bass kernel for nn_DirectInjectionEncoder (moe_routing).

Strategy (8 NeuronCores):
  - Every projection GEMM (Wgate/Wup/Wdown 10240->2560 and Wk/Wv 640->2560)
    is sharded over the output dim d_model=2560 -> 320 columns per core, so
    each core streams only 1/8 of every weight from HBM.
  - Big-group GEMMs run as e4m3 DoubleRow matmuls (2x PE rate, both
    operands fp8, K=256 per instruction). A KERNEL_F8_FRAC knob can move a
    fraction of the contraction to e3m4-stationary x bf16-moving matmuls
    (1x PE rate, higher precision) inside the same PSUM accumulation
    group. Weights are pre-scaled by 50 on the host; the row normalization
    divides the scale back out. Row norms come from fp32 PSUM
    square-accumulation, so fp8 only affects the GEMM itself.
  - The DoubleRow path streams x and W together: the host packs
    [x-rows | W-cols] into one supertile per k-chunk so each chunk is a
    single large contiguous DMA. Chunk sizes are graduated (2,6,8,...) so
    the PE's first matmul has data as early as possible.
  - Row L2-norms need the full 2560-dim row: each core computes partial
    sums of squares; two small AllReduces (~19us fixed cost each on this
    8-core mesh) combine them. AR1 covers the first two big groups and
    flies while the rest computes; AR2 covers the last big group + both kv
    groups (ordered last because they are tiny), so only one collective
    sits in the tail.
  - Identity tokens (9 of 14 slots/layer, first 2560 dims, no weights) are
    data-parallel over the batch in bf16 both ways; their loads/stores and
    ACT/DVE norm work fill PE-idle and collective windows.
  - All outputs are stored as bf16 (the 2e-2 rel-err budget dwarfs bf16
    rounding); the host casts back to fp32 when assembling.
"""

import os
import sys

sys.path.insert(0, "/opt/trn_rl_repo")

import numpy as np
import ml_dtypes

from concourse import bacc, mybir
from concourse.bass_utils import run_bass_kernel_spmd
from concourse.tile import TileContext

D_MODEL = 2560
NUM_LAYERS = 36
TOKENS_PER_LAYER = 14
EPS = 1e-8
B = 16
N_CORES = 8
CORE_IDS = list(range(N_CORES))
D_SHARD = D_MODEL // N_CORES  # 320
ROWS = B * NUM_LAYERS  # 576
XW = ROWS + D_SHARD  # 896 combined columns in the fused x|w supertiles
ROW_TILES = [(0, 128), (128, 128), (256, 128), (384, 128), (512, 64)]
NRT = len(ROW_TILES)

IDENTITY_OFFSETS = np.array([0, 1, 2, 4, 6, 7, 8, 10, 13])
# big groups first; tiny kv groups last so the tail collective covers the
# least-latency-critical work
GROUP_ORDER = [
    ("big", 11, "Wup"),
    ("big", 9, "Wgate"),
    ("big", 12, "Wdown"),
    ("kv", 3, "Wk"),
    ("kv", 5, "Wv"),
]
KV_IND = 640
BIG_IND = 10240
N_SSQ = 25  # 5 groups x 5 row tiles, in GROUP_ORDER order
AR1_COLS = 10  # big0 + big1

ID_ROWS_RAW = (B // N_CORES) * NUM_LAYERS * len(IDENTITY_OFFSETS)  # 648
ID_SUP = 2  # two supertiles of 3 row-blocks each (768 rows padded)

W_SCALE = 50.0  # folded into the normalization

F32 = mybir.dt.float32
BF16 = mybir.dt.bfloat16
F8E4 = mybir.dt.float8e4
F8E3 = mybir.dt.float8e3
DR = mybir.MatmulPerfMode.DoubleRow
AF = mybir.ActivationFunctionType

NP_BF16 = ml_dtypes.bfloat16
NP_E4 = ml_dtypes.float8_e4m3
NP_E3 = ml_dtypes.float8_e3m4

# fraction of the big-group contraction dim run as e4m3 DoubleRow
F8_FRAC = float(os.environ.get("KERNEL_F8_FRAC", "1.0"))
N_KT = BIG_IND // 128  # 80 contraction tiles of 128
N_DR = int(round(F8_FRAC * N_KT / 2))  # 256-wide DoubleRow k-tiles
N_E3 = N_KT - 2 * N_DR  # 128-wide e3m4xbf16 k-tiles
K_DR = 256 * N_DR


def _dr_chunks(n):
    # graduated chunk sizes: small first chunks cut PE startup latency
    out = []
    for c in (2, 6):
        if n >= c:
            out.append(c)
            n -= c
    while n > 8:
        out.append(8)
        n -= 8
    if n:
        out.append(n)
    return out


CHUNKS_DR = _dr_chunks(N_DR) if N_DR else []


def _pick_kb(n, cap=8):
    for kb in range(min(n, cap), 0, -1):
        if n % kb == 0:
            return kb
    return 1


KB_E3 = _pick_kb(N_E3, 8) if N_E3 else 1
NSUP_E3 = N_E3 // KB_E3 if N_E3 else 0


def _positions(offset):
    return np.arange(NUM_LAYERS) * TOKENS_PER_LAYER + offset


def build_program():
    nc = bacc.Bacc("TRN2", num_devices=N_CORES)

    big_idx = [i for i, g in enumerate(GROUP_ORDER) if g[0] == "big"]
    kv_idx = [i for i, g in enumerate(GROUP_ORDER) if g[0] == "kv"]

    xdr_d, wdr_d, xe3_d, we3_d, om_d = {}, {}, {}, {}, {}
    for gi in big_idx:
        for ci, kb in enumerate(CHUNKS_DR):
            xdr_d[(gi, ci)] = nc.declare_dram_parameter(f"xdr_{gi}_{ci}", [128, kb, 2, ROWS], F8E4, isOutput=False)
            wdr_d[(gi, ci)] = nc.declare_dram_parameter(f"wdr_{gi}_{ci}", [128, kb, 2, D_SHARD], F8E4, isOutput=False)
        if NSUP_E3:
            xe3_d[gi] = nc.declare_dram_parameter(f"xe3_{gi}", [NSUP_E3, 128, KB_E3, ROWS], F8E3, isOutput=False)
            we3_d[gi] = nc.declare_dram_parameter(f"we3_{gi}", [NSUP_E3, 128, KB_E3, D_SHARD], BF16, isOutput=False)
        om_d[gi] = nc.declare_dram_parameter(f"om_{gi}", [128, NRT * D_SHARD], BF16, isOutput=True)
    kvx_d, kvw_d, kvo_d = {}, {}, {}
    for gi in kv_idx:
        kvx_d[gi] = nc.declare_dram_parameter(f"kvx_{gi}", [128, 5, ROWS], BF16, isOutput=False)
        kvw_d[gi] = nc.declare_dram_parameter(f"kvw_{gi}", [128, 5, D_SHARD], BF16, isOutput=False)
        kvo_d[gi] = nc.declare_dram_parameter(f"kvo_{gi}", [128, NRT * D_SHARD], BF16, isOutput=True)
    idx_d = nc.declare_dram_parameter("id_x", [ID_SUP, 128, 3 * D_MODEL], BF16, isOutput=False)
    ido_d = nc.declare_dram_parameter("out_id", [ID_SUP, 128, 3 * D_MODEL], BF16, isOutput=True)

    with TileContext(nc) as tc:
        with (
            tc.tile_pool(name="xw", bufs=3) as xw_pool,
            tc.tile_pool(name="wdr", bufs=3) as wdr_pool,
            tc.tile_pool(name="xe", bufs=3) as xe_pool,
            tc.tile_pool(name="we", bufs=3) as we_pool,
            tc.tile_pool(name="sout", bufs=N_SSQ) as sout_pool,
            tc.tile_pool(name="scr", bufs=2) as scr_pool,
            tc.tile_pool(name="kvp", bufs=4) as kv_pool,
            tc.tile_pool(name="idp", bufs=2) as id_pool,
            tc.tile_pool(name="obf", bufs=6) as obf_pool,
            tc.tile_pool(name="small", bufs=1) as small_pool,
            tc.tile_pool(name="ps", bufs=8, space="PSUM") as psum_pool,
            tc.tile_pool(name="dram", bufs=1, space="DRAM") as dram_pool,
        ):
            ssq = small_pool.tile([128, N_SSQ], F32, tag="ssq")
            nc.vector.memset(ssq[:], 0.0)

            # Warmup AllReduce: the first collective pays a ~35us all-core
            # barrier plus ~30us cold cost; hide both under the GEMM phase.
            warm_sb = small_pool.tile([1, 16], F32, tag="warmsb")
            nc.vector.memset(warm_sb[:], 0.0)
            warm_buf = dram_pool.tile([16], F32, tag="warmci")
            nc.gpsimd.dma_start(out=warm_buf[:], in_=warm_sb[0, :])
            nc.gpsimd.collective_compute(
                "AllReduce",
                mybir.AluOpType.add,
                ins=[warm_buf.opt()],
                outs=[warm_buf.opt()],
                replica_groups=[CORE_IDS],
            )

            id_tiles = [None] * ID_SUP
            id_out = [None] * ID_SUP
            id_ssq = small_pool.tile([128, ID_SUP * 3], F32, tag="idssq")
            id_scale = small_pool.tile([128, ID_SUP * 3], F32, tag="idscale")

            def id_load(s):
                it = id_pool.tile([128, 3, D_MODEL], BF16, tag="idin", name=f"idin_{s}")
                nc.sync.dma_start(out=it[:], in_=idx_d[s].rearrange("p (j c) -> p j c", j=3))
                id_tiles[s] = it
                id_out[s] = id_pool.tile([128, 3, D_MODEL], BF16, tag="idout", name=f"idout_{s}")

            def id_square(s, j):
                iscr = scr_pool.tile([128, D_MODEL], BF16, tag="idscr", name=f"idscr_{s}_{j}")
                nc.scalar.activation(
                    iscr[:], id_tiles[s][:, j, :], AF.Square,
                    accum_out=id_ssq[:, s * 3 + j : s * 3 + j + 1],
                )

            def id_finish(s):
                c0 = s * 3
                nrm = small_pool.tile([128, 3], F32, tag=f"idn{s}", name=f"idnrm_{s}")
                nc.scalar.sqrt(nrm[:], id_ssq[:, c0 : c0 + 3])
                nc.scalar.activation(nrm[:], nrm[:], AF.Copy, bias=EPS)
                nc.vector.reciprocal(id_scale[:, c0 : c0 + 3], nrm[:])
                for j in range(3):
                    nc.vector.tensor_scalar_mul(
                        id_out[s][:, j, :], id_tiles[s][:, j, :],
                        id_scale[:, c0 + j : c0 + j + 1],
                    )
                nc.scalar.dma_start(
                    out=ido_d[s].rearrange("p (j c) -> p j c", j=3), in_=id_out[s][:]
                )

            kv_tiles = {}

            def kv_load():
                for gi in kv_idx:
                    kvx = kv_pool.tile([128, 5, ROWS], BF16, tag="kvx", name=f"kvx_{gi}")
                    kvw = kv_pool.tile([128, 5, D_SHARD], BF16, tag="kvw", name=f"kvw_{gi}")
                    nc.sync.dma_start(out=kvx[:], in_=kvx_d[gi][:, :, :])
                    nc.sync.dma_start(out=kvw[:], in_=kvw_d[gi][:, :, :])
                    kv_tiles[gi] = (kvx, kvw)

            souts = {}

            def big_group(gi):
                ps = [
                    psum_pool.tile([128, D_SHARD], F32, tag="ps", name=f"ps_{gi}_{r}")
                    for r in range(NRT)
                ]
                kt = 0
                for ci, kb in enumerate(CHUNKS_DR):
                    xt = xw_pool.tile([128, kb, 2, ROWS], F8E4, tag="xw", name=f"xdr_{gi}_{ci}")
                    wt = wdr_pool.tile([128, kb, 2, D_SHARD], F8E4, tag="wdr", name=f"wdr_{gi}_{ci}")
                    nc.sync.dma_start(out=xt[:], in_=xdr_d[(gi, ci)][:, :, :, :])
                    nc.sync.dma_start(out=wt[:], in_=wdr_d[(gi, ci)][:, :, :, :])
                    for j in range(kb):
                        for r, (r0, rw) in enumerate(ROW_TILES):
                            nc.tensor.matmul(
                                ps[r][:rw, :],
                                xt[:, j, :, r0 : r0 + rw],
                                wt[:, j, :, :],
                                start=(kt == 0),
                                stop=(N_E3 == 0 and kt == N_DR - 1),
                                perf_mode=DR,
                                skip_group_check=True,
                            )
                        kt += 1
                ke = 0
                for js in range(NSUP_E3):
                    xt = xe_pool.tile([128, KB_E3, ROWS], F8E3, tag="xe", name=f"xe3_{gi}_{js}")
                    wt = we_pool.tile([128, KB_E3, D_SHARD], BF16, tag="we", name=f"we3_{gi}_{js}")
                    nc.sync.dma_start(out=xt[:], in_=xe3_d[gi][js])
                    nc.sync.dma_start(out=wt[:], in_=we3_d[gi][js])
                    for j in range(KB_E3):
                        last = ke == N_E3 - 1
                        for r, (r0, rw) in enumerate(ROW_TILES):
                            nc.tensor.matmul(
                                ps[r][:rw, :],
                                xt[:, j, r0 : r0 + rw],
                                wt[:, j, :],
                                start=(N_DR == 0 and ke == 0),
                                stop=last,
                                skip_group_check=True,
                            )
                        ke += 1
                return ps

            def kv_group(gi):
                kvx, kvw = kv_tiles[gi]
                ps = [
                    psum_pool.tile([128, D_SHARD], F32, tag="ps", name=f"pkv_{gi}_{r}")
                    for r in range(NRT)
                ]
                for k in range(5):
                    for r, (r0, rw) in enumerate(ROW_TILES):
                        nc.tensor.matmul(
                            ps[r][:rw, :],
                            kvx[:, k, r0 : r0 + rw],
                            kvw[:, k, :],
                            start=(k == 0),
                            stop=(k == 4),
                        )
                return ps

            def drain(slot, ps):
                # copy psum->bf16 sbuf + square-accum into ssq columns
                for r, (r0, rw) in enumerate(ROW_TILES):
                    col = slot * NRT + r
                    scr = scr_pool.tile([128, D_SHARD], BF16, tag="scr", name=f"scr_{slot}_{r}")
                    nc.scalar.activation(
                        scr[:rw, :], ps[r][:rw, :], AF.Square,
                        accum_out=ssq[:rw, col : col + 1],
                    )
                    so = sout_pool.tile([128, D_SHARD], BF16, tag="sout", name=f"so_{slot}_{r}")
                    nc.vector.tensor_copy(so[:rw, :], ps[r][:rw, :])
                    souts[(slot, r)] = so

            def scale_store(slot, sc, sc_col0):
                kind = GROUP_ORDER[slot][0]
                ob = obf_pool.tile([128, NRT, D_SHARD], BF16, tag="obf", name=f"obf_{slot}")
                for r, (r0, rw) in enumerate(ROW_TILES):
                    col = slot * NRT + r - sc_col0
                    src = souts[(slot, r)]
                    if r % 2 == 0:
                        nc.vector.tensor_scalar_mul(
                            ob[:rw, r, :], src[:rw, :], sc[:rw, col : col + 1]
                        )
                    else:
                        nc.scalar.activation(
                            ob[:rw, r, :], src[:rw, :], AF.Copy,
                            scale=sc[:rw, col : col + 1],
                        )
                dst = om_d[slot] if kind == "big" else kvo_d[slot]
                nc.scalar.dma_start(
                    out=dst.rearrange("p (r c) -> p r c", r=NRT), in_=ob[:]
                )

            def reduce_scale(ar_buf, ncols, col0):
                # readback of the AllReduce result on the gpsimd ring, then
                # norm math on ACT/DVE
                tsq = small_pool.tile([128, ncols], F32, tag=f"tsq{col0}", name=f"tsq_{col0}")
                nc.gpsimd.dma_start(out=tsq[:], in_=ar_buf[:])
                nrm = small_pool.tile([128, ncols], F32, tag=f"nrm{col0}", name=f"nrm_{col0}")
                nc.scalar.sqrt(nrm[:], tsq[:])
                nc.scalar.activation(nrm[:], nrm[:], AF.Copy, bias=EPS)
                sc = small_pool.tile([128, ncols], F32, tag=f"sc{col0}", name=f"sc_{col0}")
                nc.vector.reciprocal(sc[:], nrm[:])
                return sc

            def all_reduce(cols0, cols1, tag):
                buf = dram_pool.tile([128, cols1 - cols0], F32, tag=f"ar{tag}")
                nc.gpsimd.dma_start(out=buf[:], in_=ssq[:, cols0:cols1])
                nc.gpsimd.collective_compute(
                    "AllReduce",
                    mybir.AluOpType.add,
                    ins=[buf.opt()],
                    outs=[buf.opt()],
                    replica_groups=[CORE_IDS],
                )
                return buf

            # ---- main pipeline ----
            ar1 = None
            for slot, (kind, off, wname) in enumerate(GROUP_ORDER):
                ps = big_group(slot) if kind == "big" else kv_group(slot)
                drain(slot, ps)
                if slot == 0:
                    id_load(0)
                    id_square(0, 0)
                    id_square(0, 1)
                    id_square(0, 2)
                    id_finish(0)
                elif slot == 1:
                    # AR1: big0 + big1 norm partials fly during big2/kv
                    ar1 = all_reduce(0, AR1_COLS, "1")
                    kv_load()
                    id_load(1)
                    id_square(1, 0)
                    id_square(1, 1)
                    id_square(1, 2)
                    id_finish(1)
                elif slot == 2:
                    # AR1 is long done by the time ACT reaches this point;
                    # emitting the norm math here keeps it ahead of the kv
                    # squares in the ACT queue.
                    sc1 = reduce_scale(ar1, AR1_COLS, 0)
                    scale_store(0, sc1, 0)
                    scale_store(1, sc1, 0)

            ar2 = all_reduce(AR1_COLS, N_SSQ, "2")

            sc2 = reduce_scale(ar2, N_SSQ - AR1_COLS, AR1_COLS)
            for slot in range(2, 5):
                scale_store(slot, sc2, AR1_COLS)

    nc.compile()
    return nc


_NC = None


def _get_nc():
    global _NC
    if _NC is None:
        _NC = build_program()
    return _NC


def _pack_sup_e3(xT):
    # [K, C] -> [nsup, 128, kb, C]; k = ((js*kb + j)*128 + p)
    K, C = xT.shape
    nsup = K // (128 * KB_E3)
    return np.ascontiguousarray(xT.reshape(nsup, KB_E3, 128, C).transpose(0, 2, 1, 3))


def _pack_chunks_dr(aT):
    # [K_DR, C] -> per-chunk [128, kb, 2, C]; k = ((base + j)*2 + i)*128 + p
    out = []
    base = 0
    C = aT.shape[1]
    for kb in CHUNKS_DR:
        a = aT[base * 256 : (base + kb) * 256]
        out.append(np.ascontiguousarray(a.reshape(kb, 2, 128, C).transpose(2, 0, 1, 3)))
        base += kb
    return out


def _pack_kv(xT):
    # [640, C] -> [128, 5, C]
    K, C = xT.shape
    return np.ascontiguousarray(xT.reshape(5, 128, C).transpose(1, 0, 2))


def _prep_inputs(lora_tokens, weights):
    lora = np.ascontiguousarray(lora_tokens)
    big_idx = [(i, g[1], g[2]) for i, g in enumerate(GROUP_ORDER) if g[0] == "big"]
    kv_idx = [(i, g[1], g[2]) for i, g in enumerate(GROUP_ORDER) if g[0] == "kv"]

    shared = {}
    x_chunks = {}
    for gi, off, wname in big_idx:
        pos = _positions(off)
        x = lora[:, pos, :].reshape(ROWS, BIG_IND).T  # [10240, 576]
        if N_DR:
            x_chunks[gi] = _pack_chunks_dr(np.clip(x[:K_DR], -240, 240).astype(NP_E4))
        if N_E3:
            shared[f"xe3_{gi}"] = _pack_sup_e3(np.clip(x[K_DR:] * 2.0, -15.0, 15.0).astype(NP_E3))
    kv_xt = {}
    for gi, off, wname in kv_idx:
        pos = _positions(off)
        kv_xt[gi] = lora[:, pos, :KV_IND].reshape(ROWS, KV_IND).T  # [640, 576]
        shared[f"kvx_{gi}"] = _pack_kv(kv_xt[gi].astype(NP_BF16))

    id_pos = np.sort(np.concatenate([_positions(o) for o in IDENTITY_OFFSETS]))
    bpc = B // N_CORES
    in_maps = []
    for c in range(N_CORES):
        m = dict(shared)
        csl = slice(c * D_SHARD, (c + 1) * D_SHARD)
        for gi, off, wname in big_idx:
            wT = weights[wname][csl, :].T  # [10240, 320]
            if N_DR:
                for ci, chunk in enumerate(x_chunks[gi]):
                    m[f"xdr_{gi}_{ci}"] = chunk
                w8 = np.clip(wT[:K_DR] * W_SCALE, -240, 240).astype(NP_E4)
                for ci, chunk in enumerate(_pack_chunks_dr(w8)):
                    m[f"wdr_{gi}_{ci}"] = chunk
            if N_E3:
                m[f"we3_{gi}"] = _pack_sup_e3((wT[K_DR:] * (W_SCALE / 2.0)).astype(NP_BF16))
        for gi, off, wname in kv_idx:
            m[f"kvw_{gi}"] = _pack_kv(weights[wname][csl, :].T.astype(NP_BF16))
        idx = lora[c * bpc : (c + 1) * bpc, :, :][:, id_pos, :D_MODEL].reshape(
            ID_ROWS_RAW, D_MODEL
        )
        idp = np.ones((ID_SUP * 3 * 128, D_MODEL), dtype=np.float32)
        idp[:ID_ROWS_RAW] = idx
        # row = (s*3 + j)*128 + p  ->  [ID_SUP, 128, 3*D_MODEL]
        m["id_x"] = np.ascontiguousarray(
            idp.reshape(ID_SUP, 3, 128, D_MODEL).transpose(0, 2, 1, 3).reshape(
                ID_SUP, 128, 3 * D_MODEL
            ).astype(NP_BF16)
        )
        in_maps.append(m)
    return in_maps, id_pos


def _unpack_rows(arr128, nrt=NRT, width=D_SHARD):
    # [128, nrt*width] (bf16) -> [nrt*128, width] fp32, caller trims rows
    a = np.asarray(arr128).astype(np.float32).reshape(128, nrt, width)
    return a.transpose(1, 0, 2).reshape(nrt * 128, width)


def run(inputs, trace=False):
    nc = _get_nc()
    weights = {k: inputs[k] for k in ("Wk", "Wv", "Wgate", "Wup", "Wdown")}
    in_maps, id_pos = _prep_inputs(inputs["lora_tokens"], weights)
    res = run_bass_kernel_spmd(nc, in_maps, CORE_IDS, trace=trace)

    out = np.zeros((B, NUM_LAYERS * TOKENS_PER_LAYER, D_MODEL), dtype=np.float32)
    bpc = B // N_CORES
    for c in range(N_CORES):
        r = res.results[c]
        csl = slice(c * D_SHARD, (c + 1) * D_SHARD)
        for slot, (kind, off, wname) in enumerate(GROUP_ORDER):
            pos = _positions(off)
            key = f"om_{slot}" if kind == "big" else f"kvo_{slot}"
            rows = _unpack_rows(r[key])[:ROWS]
            out[:, pos, csl] = rows.reshape(B, NUM_LAYERS, D_SHARD)
        ido = np.asarray(r["out_id"]).astype(np.float32).reshape(ID_SUP, 128, 3, D_MODEL)
        ido = ido.transpose(0, 2, 1, 3).reshape(ID_SUP * 3 * 128, D_MODEL)[:ID_ROWS_RAW]
        out[c * bpc : (c + 1) * bpc, id_pos, :] = ido.reshape(bpc, len(id_pos), D_MODEL)
    return out, res


def kernel(**inputs) -> np.ndarray:
    out, _ = run(inputs, trace=False)
    return out
